# revision 24
# baseline (speedup 1.0000x reference)
"""DiT attention kernel for Trainium2 (Bass/Tile), data-parallel over batch.

Problem: B=8, S=1024, D=1024, H=16 heads, head_dim=64, fp32.
  q = x@wq.T; k = x@wk.T; v = x@wv.T          (per batch)
  attn = softmax(q k^T / sqrt(hd)); out = (attn v) @ wo.T

Sharding: batch is split 1:1 onto the 8 NeuronCores (pure data parallel,
no collectives). Weights are broadcast. Host pre-transposes x (per batch)
and the four weights so every matmul has its contraction dim on SBUF
partitions; all matmuls run as float32r (full-rate fp32, ~1e-4 rel err).

v2 over the original pipeline: the kernel is PE-row-bound (f32r moving
rows at 1 cycle/row; fp8 double-pumping is numerically out of budget),
so all changes target PE idle gaps and p-state ramp resets:
  - warmup matmuls hold the PE clock at full rate until the first x/wv
    DMA lands; the first V matmul runs on quarter-size x/wv pieces.
  - V-pass PSUM copies are interleaved into the last contraction chunk
    so the next pass's bank reuse never stalls.
  - wq0/wk0 are DMA'd during the V pass.
  - exp runs as two 512-wide halves so attnV(kc) can start on half an
    exp tile; each head's last attnV + drain (stage copy, sumexp/raw
    DMAs) is deferred into the NEXT head's kc=1 slot so PE never waits
    on ACT at head boundaries.
  - wo is prefetched whole into dead QT/KT slots during oc=3..6.
  - the output projection is sc-outer (per-chunk copy+DMA right after
    each chain) and the last pair's softmax normalization is rebuilt
    without the partition-shift DMA: per-head reciprocal of the sumexp
    row + a K=1 ones-row broadcast matmul, interleaved into the first
    output chain, so the projection never stalls on the last norm.
"""
import numpy as np
from contextlib import ExitStack

import concourse.bass as bass
import concourse.mybir as mybir
import concourse.tile as tile
from concourse import bacc
import concourse.bass_utils as bass_utils
from concourse.bass import ds

B, S, D, H = 8, 1024, 1024, 16
HD = D // H          # 64
P = 128
NCORES = 8
DC = D // P          # 8 chunks of the feature dim
SC = S // P          # 8 chunks of the sequence dim
NH = 512             # matmul moving-dim chunk (fp32 limit, one PSUM bank)

f32 = mybir.dt.float32
f32r = mybir.dt.float32r
AF = mybir.ActivationFunctionType
ALU = mybir.AluOpType

N_WARMUP = 10        # PE warmup matmuls (tuned to first-DMA latency)


def emit(tc, xT_d, wqT_d, wkT_d, wvT_d, woT_d, y_d):
    nc = tc.nc
    with ExitStack() as ctx:
        xp = ctx.enter_context(tc.tile_pool(name="xp", bufs=1))
        qkp = ctx.enter_context(tc.tile_pool(name="qkp", bufs=1))
        vp = ctx.enter_context(tc.tile_pool(name="vp", bufs=1))
        ep = ctx.enter_context(tc.tile_pool(name="ep", bufs=4))
        rp = ctx.enter_context(tc.tile_pool(name="rp", bufs=1))
        stp = ctx.enter_context(tc.tile_pool(name="stp", bufs=1))
        sxq = ctx.enter_context(tc.tile_pool(name="sxq", bufs=2))
        sxp = ctx.enter_context(tc.tile_pool(name="sxp", bufs=1))
        wp = ctx.enter_context(tc.tile_pool(name="wp", bufs=3))
        wrp = ctx.enter_context(tc.tile_pool(name="wrp", bufs=3))
        yp = ctx.enter_context(tc.tile_pool(name="yp", bufs=2))
        pp = ctx.enter_context(tc.tile_pool(name="pp", bufs=4, space="PSUM"))

        # ---- PE warmup: hold the clock at full p-state until the first
        # x/wv DMAs land (the cost model halves PE rate for 3us after any
        # idle->busy transition). Runs off the memset selector tile.
        sel2_f = wp.tile([2 * H, P], f32, tag="wqk")
        nc.vector.memset(sel2_f[:], 1.0)
        psW = pp.tile([P, 2 * NH], f32, tag="ps", name="psW")
        for i in range(N_WARMUP):
            nc.tensor.matmul(psW[0:P, 0:P], sel2_f[:], sel2_f[:],
                             start=True, stop=True)

        # ---- V projection: V_aug [s_part, sc, head, 65] ----
        V = vp.tile([P, SC, H, HD + 1], f32r, tag="v")
        ones_t = yp.tile([P, H], f32, tag="y")
        nc.vector.memset(ones_t[:], 1.0)
        for sc in range(SC):
            nc.vector.tensor_copy(V[:, sc, :, HD], ones_t[:])

        def load_wqk(oc, key, wd):
            wt = wp.tile([P, DC, P], f32r, tag="wqk", name=f"w{key}{oc}")
            nc.sync.dma_start(wt[:], wd[oc])
            return wt

        xts = []

        def emit_v_pass(oh, early_wqk=None):
            psVs = [pp.tile([P, 2 * NH], f32, tag="ps", name=f"psV{oh}_{j}")
                    for j in range(4)]
            copied = [False] * 4

            def vcopy(j):
                src = psVs[j]
                for half in range(2):
                    s_ap = src[:, ds(half * NH, NH)]
                    dst = V[:, 2 * j + half, ds(oh * 8, 8), 0:HD]
                    if (2 * j + half) % 2 == 0:
                        nc.vector.tensor_copy(
                            dst, s_ap.rearrange("p (h e) -> p h e", e=HD))
                    else:
                        nc.scalar.copy(
                            dst, s_ap.rearrange("p (h e) -> p h e", e=HD))
                copied[j] = True

            for dc in range(DC):
                wvt = wrp.tile([P, NH], f32r, tag="wr", name=f"wv{oh}_{dc}")
                if oh == 0 and dc == 0:
                    # split first loads so the very first matmul waits for
                    # the smallest possible DMA footprint (subtile deps)
                    nc.sync.dma_start(wvt[:, 0:NH // 2], wvT_d[ds(0, P), 0:NH // 2])
                    nc.sync.dma_start(wvt[:, NH // 2:NH],
                                      wvT_d[ds(0, P), ds(NH // 2, NH // 2)])
                    t = xp.tile([P, S], f32r, tag="x0", name="x0")
                    nc.sync.dma_start(t[:, 0:P], xT_d[ds(0, P), 0:P])
                    nc.sync.dma_start(t[:, P:NH], xT_d[ds(0, P), P:NH])
                    nc.sync.dma_start(t[:, NH:S], xT_d[ds(0, P), NH:S])
                    xts.append(t)
                    # wq0/wk0 land during the V pass so Q/K never wait
                    wqk_first = (load_wqk(0, "q", wqT_d),
                                 load_wqk(0, "k", wkT_d))
                else:
                    nc.sync.dma_start(wvt[:], wvT_d[ds(dc * P, P), ds(oh * NH, NH)])
                    if oh == 0:
                        t = xp.tile([P, S], f32r, tag=f"x{dc}", name=f"x{dc}")
                        nc.sync.dma_start(t[:, 0:NH], xT_d[ds(dc * P, P), 0:NH])
                        nc.sync.dma_start(t[:, NH:S], xT_d[ds(dc * P, P), NH:S])
                        xts.append(t)
                last = dc == DC - 1
                for sc in range(SC):
                    if oh == 0 and dc == 0:
                        for hv in range(2):
                            nc.tensor.matmul(
                                psVs[sc // 2][:, ds((sc % 2) * NH + hv * (NH // 2), NH // 2)],
                                xts[0][:, ds(sc * P, P)],
                                wvt[:, ds(hv * (NH // 2), NH // 2)],
                                start=True, stop=False)
                        continue
                    nc.tensor.matmul(
                        psVs[sc // 2][:, ds((sc % 2) * NH, NH)],
                        xts[dc][:, ds(sc * P, P)], wvt[:],
                        start=(oh == 1 and dc == 0), stop=last)
                    # interleave the drain copies into the last chunk so the
                    # next pass's PSUM reuse never waits
                    if last and sc % 2 == 1:
                        vcopy(sc // 2)
            for j in range(4):
                if not copied[j]:
                    vcopy(j)
            return wqk_first if oh == 0 else None

        # oh=0 pass: dc=0 contributes via start=True (split); dc>=1 accumulate.
        # Fix start flags: dc==0 did start=True; others must not restart.
        wqk_first = emit_v_pass(0)
        emit_v_pass(1)

        # ---- softmax-denominator spread selector (pairs 0..6) ----
        nc.gpsimd.affine_select(
            out=sel2_f[:].rearrange("k (p2 m) -> k p2 m", m=HD),
            in_=sel2_f[:].rearrange("k (p2 m) -> k p2 m", m=HD),
            compare_op=ALU.is_equal,
            fill=0.0,
            base=0,
            pattern=[[-1, 2], [0, HD]],
            channel_multiplier=1,
        )
        sel2 = sxp.tile([2 * H, P], f32r, tag="on")
        nc.vector.tensor_copy(sel2[:], sel2_f[:])
        ones64 = sxp.tile([HD + 1, HD], f32r, tag="o64")
        nc.vector.memset(ones64[:], 1.0)

        # ---- software-pipelined Q/K projection + attention ----
        QT, KT, raws = {}, {}, {}

        def qk_gen(oc, key, wd, store, wt=None):
            if wt is None:
                wt = load_wqk(oc, key, wd)
            ps = pp.tile([P, 2 * NH], f32, tag="ps", name=f"ps{key}{oc}")
            for dc in range(DC):
                for sh in range(2):
                    nc.tensor.matmul(
                        ps[:, ds(sh * NH, NH)], wt[:, dc, :],
                        xts[dc][:, ds(sh * NH, NH)],
                        start=(dc == 0), stop=(dc == DC - 1))
                yield
            dst = qkp.tile([P, S], f32r, tag=f"{key}{oc}", name=f"t{key}{oc}")
            nc.vector.tensor_copy(dst[:], ps[:])
            store[oc] = dst

        def emit_qk(oc, key, wd, store, wt=None):
            for _ in qk_gen(oc, key, wd, store, wt=wt):
                pass

        # last-pair normalization scratch (no partition-shift DMA for oc=7):
        # per-head reciprocal rows, broadcast by a K=1 ones matmul.
        # rows hh*HD hold the last pair's reciprocal sumexp (partition base
        # 0/64 keeps the broadcast matmul operands legal); allocated lazily
        # into KT(4)'s slot once that tile is fully consumed
        r7box = {}

        def r7r():
            if "t" not in r7box:
                r7box["t"] = qkp.tile([HD + 1, S], f32r, tag="k4", name="r7r")
            return r7box["t"]

        def emit_head(oc, hh, rawt, sxpair, filler=None, pending=None):
            """Emit one head's scores+exp+attnV. Returns a closure that
            finishes the head (last attnV, stage copy, sumexp/raw DMA) --
            the caller fires it inside the NEXT head at kc==1."""
            h = 2 * oc + hh
            psO = pp.tile([P, 2 * NH], f32, tag="ps", name=f"psO{h}")
            ets = {}

            def attn_v(kc):
                for qh in range(2):
                    nc.tensor.matmul(
                        psO[0:HD + 1, ds(qh * NH, NH)],
                        V[:, kc, h, :], ets[kc][:, ds(qh * NH, NH)],
                        start=(kc == 0), stop=(kc == SC - 1))

            for kc in range(SC):
                psS = pp.tile([P, 2 * NH], f32, tag="ps", name=f"psS{h}_{kc}")
                lhsT = KT[oc][ds(hh * HD, HD), ds(kc * P, P)]
                for qh in range(2):
                    nc.tensor.matmul(
                        psS[:, ds(qh * NH, NH)], lhsT,
                        QT[oc][ds(hh * HD, HD), ds(qh * NH, NH)],
                        start=True, stop=True)
                et = ep.tile([P, S], f32r, tag="e", name=f"et{h}_{kc}")
                # two half-width exps: attnV(kc) can start on the first half
                nc.scalar.activation(et[:, 0:NH], psS[:, 0:NH],
                                     AF.Exp, scale=0.125)
                nc.scalar.activation(et[:, NH:S], psS[:, NH:S],
                                     AF.Exp, scale=0.125)
                ets[kc] = et
                if kc > 0:
                    attn_v(kc - 1)
                if kc == 1 and pending is not None:
                    pending()
                if filler is not None:
                    next(filler, None)
            if filler is not None:
                for _ in filler:
                    pass

            def finish():
                attn_v(SC - 1)
                stage = stp.tile([HD + 1, S], f32r, tag="st", name=f"stage{h}")
                # sumexp row first so the norm chain starts early
                nc.vector.tensor_copy(stage[ds(HD, 1), :], psO[ds(HD, 1), :])
                if oc == DC - 1:
                    # last pair: reciprocal straight off the stage row; the
                    # broadcast happens via a K=1 matmul in the Y phase
                    nc.vector.reciprocal_approx_fast(
                        out=r7r()[ds(hh * HD, 1), :].bitcast(f32),
                        in_=stage[ds(HD, 1), :].bitcast(f32))
                else:
                    nc.sync.dma_start(sxpair[ds(hh, 1), :], stage[ds(HD, 1), :])
                nc.vector.tensor_copy(stage[0:HD, :], psO[0:HD, :])
                nc.sync.dma_start(rawt[ds(hh * HD, HD), :], stage[0:HD, :])

            return finish

        sxpairs = {}

        def emit_norm(oc):
            sxpair = sxpairs[oc]
            # QT(oc) died with this pair's heads; reuse its slot as scratch
            scratch = qkp.tile([2 * H, S], f32, tag=f"q{oc}", name=f"rcs{oc}")
            nc.vector.reciprocal_approx_fast(
                out=scratch[:], in_=sxpair[:].bitcast(f32))
            nc.vector.tensor_copy(sxpair[:], scratch[:])
            psB = pp.tile([P, 2 * NH], f32, tag="ps", name=f"psB{oc}")
            for qh in range(2):
                nc.tensor.matmul(
                    psB[:, ds(qh * NH, NH)],
                    sel2[:], sxpair[:, ds(qh * NH, NH)],
                    start=True, stop=True)
            nc.vector.tensor_tensor(raws[oc][:], raws[oc][:], psB[:], ALU.mult)

        # wo tiles land in dead QT/KT slots (loaded whole rows, both halves)
        wots = {}

        def load_wo(i):
            tag = ["q0", "k0", "q1", "k1", "q2", "k2", "q3", "k3"][i]
            t = qkp.tile([P, S], f32r, tag=tag, name=f"wo{i}")
            nc.sync.dma_start(t[:], woT_d[ds(i * P, P), :])
            wots[i] = t

        emit_qk(0, "q", wqT_d, QT, wt=wqk_first[0])
        emit_qk(0, "k", wkT_d, KT, wt=wqk_first[1])
        pending = None
        for oc in range(DC):
            rawt = rp.tile([P, S], f32r, tag=f"r{oc}")
            raws[oc] = rawt
            if oc < DC - 1:
                sxpair = sxq.tile([2 * H, S], f32r, tag="sx", name=f"sx{oc}")
                nc.vector.tensor_copy(
                    sxpair[:], ones_t[0:2 * H, 0:1].to_broadcast((2 * H, S)))
                sxpairs[oc] = sxpair
            else:
                sxpair = None
            fq = qk_gen(oc + 1, "q", wqT_d, QT) if oc + 1 < DC else None
            pending = emit_head(oc, 0, rawt, sxpair, filler=fq, pending=pending)
            fk = qk_gen(oc + 1, "k", wkT_d, KT) if oc + 1 < DC else None
            pending = emit_head(oc, 1, rawt, sxpair, filler=fk, pending=pending)
            if oc >= 1 and oc < DC - 1:
                emit_norm(oc - 1)
            if oc >= 3 and oc <= 6:
                # wo tiles land in slots whose QT/KT (and any same-slot norm
                # scratch) finished reading two pairs ago
                load_wo(2 * (oc - 3))
                load_wo(2 * (oc - 3) + 1)

        # norm(6) was deferred past the oc loop (its sxpair lands during
        # head(7,0)); emit it before the tail so only pair 7 is special.
        emit_norm(DC - 2)

        # ---- output projection Y[s, o], sc-outer ----
        # pending() finishes head(7,1): last attnV + stage + r7 recip.
        pending()

        def norm7_spread():
            psB = pp.tile([P, 2 * NH], f32, tag="ps", name="psB7")
            for hh in range(2):
                for qh in range(2):
                    nc.tensor.matmul(
                        psB[ds(hh * HD, HD), ds(qh * NH, NH)],
                        ones64[ds(hh * HD, 1), :],
                        r7r()[ds(hh * HD, 1), ds(qh * NH, NH)],
                        start=True, stop=True)
            return psB

        psB7 = None
        for sc in range(SC):
            psY = pp.tile([P, 2 * NH], f32, tag="ps", name=f"psY{sc}")
            for dc in range(DC):
                if sc == 0 and dc == 5:
                    psB7 = norm7_spread()
                if sc == 0 and dc == 6:
                    # normalize raws[7] in quarter slices so the first Y
                    # chain's dc=7 matmul waits only on slice 0
                    for pc in range(4):
                        nc.vector.tensor_tensor(
                            raws[DC - 1][:, ds(pc * 256, 256)],
                            raws[DC - 1][:, ds(pc * 256, 256)],
                            psB7[:, ds(pc * 256, 256)], ALU.mult)
                for oh in range(2):
                    nc.tensor.matmul(
                        psY[:, ds(oh * NH, NH)],
                        raws[dc][:, ds(sc * P, P)],
                        wots[dc][:, ds(oh * NH, NH)],
                        start=(dc == 0), stop=(dc == DC - 1))
            yt = xp.tile([P, S], f32, tag=f"x{sc}", name=f"yt{sc}")
            if sc % 2 == 0:
                nc.vector.tensor_copy(yt[:, 0:S], psY[:])
            else:
                nc.scalar.copy(yt[:, 0:S], psY[:])
            nc.sync.dma_start(y_d[ds(sc * P, P), :], yt[:, 0:S])


def build_nc():
    nc = bacc.Bacc("TRN2", target_bir_lowering=False, debug=False,
                   enable_asserts=False, num_devices=NCORES)
    xT_d = nc.dram_tensor("xT", (D, S), f32r, kind="ExternalInput").ap()
    wqT_d = nc.dram_tensor("wqT", (DC, P, DC, P), f32r, kind="ExternalInput").ap()
    wkT_d = nc.dram_tensor("wkT", (DC, P, DC, P), f32r, kind="ExternalInput").ap()
    wvT_d = nc.dram_tensor("wvT", (D, D), f32r, kind="ExternalInput").ap()
    woT_d = nc.dram_tensor("woT", (D, D), f32r, kind="ExternalInput").ap()
    y_d = nc.dram_tensor("y", (S, D), f32, kind="ExternalOutput").ap()
    with tile.TileContext(nc) as tc:
        emit(tc, xT_d, wqT_d, wkT_d, wvT_d, woT_d, y_d)
    nc.compile()
    return nc


_NC_CACHE = None


def _get_nc():
    global _NC_CACHE
    if _NC_CACHE is None:
        _NC_CACHE = build_nc()
    return _NC_CACHE


def _block_qk(w):
    wT = np.asarray(w, np.float32).T
    return np.ascontiguousarray(
        wT.reshape(DC, P, DC, P).transpose(2, 1, 0, 3))


def make_in_maps(x, wq, wk, wv, wo):
    x = np.asarray(x, dtype=np.float32)
    wqT = _block_qk(wq)
    wkT = _block_qk(wk)
    wvT = np.ascontiguousarray(np.asarray(wv, np.float32).T)
    woT = np.ascontiguousarray(np.asarray(wo, np.float32).T)
    in_maps = []
    for b in range(B):
        in_maps.append({
            "xT": np.ascontiguousarray(x[b].T),
            "wqT": wqT, "wkT": wkT, "wvT": wvT, "woT": woT,
        })
    return in_maps


def kernel(x, wq, wk, wv, wo):
    nc = _get_nc()
    in_maps = make_in_maps(x, wq, wk, wv, wo)
    res = bass_utils.run_bass_kernel_spmd(nc, in_maps, core_ids=list(range(NCORES)))
    return np.stack([res.results[b]["y"] for b in range(B)], axis=0)
